# revision 37
# baseline (speedup 1.0000x reference)
"""Compressed MoE block on 8 Trainium2 NeuronCores.

Expert-parallel sharding: core e owns expert e. The router (tiny: T x H @
H x E) runs on host as part of dispatch; tokens are gathered per selected
expert (top-2), padded to a fixed capacity C, and each core runs the full
factored FFN chain for its expert in token-transposed layout:

    g1T = Ug'(e).T @ xT          (Ug' = Ug @ Cg folded on host)
    gT  = Vg(e).T  @ g1T
    u1T = Uu'(e).T @ xT
    uT  = Vu(e).T  @ u1T
    aT  = silu(gT) * uT
    d1T = Ud'(e).T @ aT          (Ud' = Ud @ Cd)
    yT  = Vd(e).T  @ d1T

Everything on device is bfloat16 (fp32 PSUM accumulate), which halves the
HBM input stream (~7 MB/core) and runs the PE at the full 1-cycle/column
rate. Phase B is split: B1 computes aT for all F tiles into SBUF with
gate/up PSUM tiles double-buffered across f (no PE stall on the ACT/DVE
consumers), then B2 accumulates d1T over all F tiles as a pure PE pass.
PSUM tiles span `nch` banks ([128, nch, 512] fp32) so the silu / mul /
PSUM->SBUF copies each cover all chunks in one wide instruction.
"""

import numpy as np
import ml_dtypes

import concourse.bacc as bacc
import concourse.mybir as mybir
import concourse.tile as tile
from concourse.bass_utils import run_bass_kernel_spmd

F32 = mybir.dt.float32
BF16 = mybir.dt.bfloat16
BF16NP = ml_dtypes.bfloat16

E = 8
KTOP = 2
H = 1024
FF = 2816
R = 256
KH = H // 128    # 8
KR = R // 128    # 2
KF = FF // 128   # 22
MH = H // 128    # 8

_BUILD_CACHE = {}
LAST_RESULT = None


def _build(C, nch):
    """Per-core bass program for capacity C split into nch chunks."""
    chunk = C // nch
    AB = 2 * R + C      # per-k block in abuf: [ugc_k | uuc_k | xt_k]
    WB = 3 * R          # per-f block in wbuf: [vg_f | vu_f | udc_f]
    NFP = (KF + 1) // 2  # f-pair DMA pieces
    nc = bacc.Bacc()

    abuf = nc.declare_dram_parameter("abuf", [KH, 128, AB], BF16, isOutput=False)
    wbuf = nc.declare_dram_parameter("wbuf", [NFP, 128, 2 * WB], BF16, isOutput=False)
    vdp = nc.declare_dram_parameter("vdp", [128, MH * R], BF16, isOutput=False)
    ytp = nc.declare_dram_parameter("ytp", [128, MH * C], BF16, isOutput=True)

    with tile.TileContext(nc) as tc:
        with (
            tc.tile_pool(name="wsb", bufs=1) as wsb,
            tc.tile_pool(name="work", bufs=3) as work,
            tc.tile_pool(name="pmm", bufs=8 // nch, space="PSUM") as pmm,
        ):
            ab = wsb.tile([128, KH, AB], BF16, tag="ab")
            wb = wsb.tile([128, KF, WB], BF16, tag="wb")
            vds = wsb.tile([128, MH * R], BF16, tag="vds")
            g1s = wsb.tile([128, KR, C], BF16, tag="g1s")
            u1s = wsb.tile([128, KR, C], BF16, tag="u1s")
            d1s = wsb.tile([128, KR, C], BF16, tag="d1s")
            afs = wsb.tile([128, KF, C], BF16, tag="afs")
            warm0 = wsb.tile([128, 256], F32, tag="warm0")
            warm = wsb.tile([128, 256], BF16, tag="warm")
            wsil = wsb.tile([128, 16], BF16, tag="wsil")

            def ugc_k(k, m):
                o = m * 128
                return ab[:, k, o:o + 128]

            def uuc_k(k, m):
                o = R + m * 128
                return ab[:, k, o:o + 128]

            def xt_k(k, c0):
                o = 2 * R + c0
                return ab[:, k, o:o + chunk]

            def vg_f(f, k):
                return wb[:, f, k * 128:(k + 1) * 128]

            def vu_f(f, k):
                return wb[:, f, R + k * 128:R + (k + 1) * 128]

            def udc_f(f, m):
                return wb[:, f, 2 * R + m * 128:2 * R + (m + 1) * 128]

            # --- PE warm-up: spin the PE from t~0 until the first abuf block
            # lands so the DVFS ramp completes during the DMA latency, and
            # preload the Silu table on ACT.
            nc.vector.memset(warm0[:], 0.0)
            nc.vector.tensor_copy(warm[:], warm0[:])
            nc.scalar.activation(
                wsil[:], warm[:, :16], mybir.ActivationFunctionType.Silu
            )
            wps = pmm.tile([128, nch, 512], F32, tag="mm", name="wps")
            NWARM = 8
            for i in range(NWARM):
                nc.tensor.matmul(
                    wps[:, 0, :16], warm[:, :128], warm[:, :16],
                    start=(i == 0), stop=(i == NWARM - 1),
                )

            # --- input DMAs. abuf alternates between the SP and ACT HWDGE
            # rings; wbuf + vdp follow on the SP ring (the ACT queue stays
            # free for silu work).
            for k in range(KH):
                eng = nc.sync if k % 2 == 0 else nc.scalar
                eng.dma_start(ab[:, k, :], abuf[k])
            for i in range(NFP):
                nc.sync.dma_start(wb[:, 2 * i:2 * i + 2, :], wbuf[i])
            nc.sync.dma_start(vds[:], vdp[:])

            # --- phase A: g1T/u1T [R, C] = Ug'/Uu'.T @ xT. k-outer with all
            # (tensor, m, chunk) accumulators concurrent, pacing the k-block
            # DMA stream; PSUM->SBUF copies at the end split across ACT/DVE.
            psA = [
                pmm.tile([128, nch, 512], F32, tag="mm", name=f"psA_{t}_{m}")
                for t in range(2) for m in range(KR)
            ]
            for k in range(KH):
                for t, wfun in enumerate((ugc_k, uuc_k)):
                    for m in range(KR):
                        for n in range(nch):
                            nc.tensor.matmul(
                                psA[t * KR + m][:, n, :chunk],
                                wfun(k, m),
                                xt_k(k, n * chunk),
                                start=(k == 0), stop=(k == KH - 1),
                            )
            for m in range(KR):
                nc.scalar.activation(
                    g1s[:, m, :], psA[m][:, :, :chunk],
                    mybir.ActivationFunctionType.Copy,
                )
            for m in range(KR):
                nc.vector.tensor_copy(u1s[:, m, :], psA[KR + m][:, :, :chunk])

            # --- phase B1: aT[f] = silu(Vg_f.T @ g1T) * (Vu_f.T @ u1T) for
            # all f, staged to SBUF. Software-pipelined with a one-f
            # stagger: each step runs gate matmuls for f=i and up matmuls
            # for f=i-1, so silu(i) (ACT) and mul(i-1) (DVE) overlap PE work
            # and PSUM banks recycle without stalling the PE.
            gsils = [None] * KF
            for i in range(KF + 1):
                if i < KF:
                    gps = pmm.tile([128, nch, 512], F32, tag="mm",
                                   name=f"gps_{i}")
                    for k in range(KR):
                        for n in range(nch):
                            c0 = n * chunk
                            nc.tensor.matmul(
                                gps[:, n, :chunk], vg_f(i, k),
                                g1s[:, k, c0:c0 + chunk],
                                start=(k == 0), stop=(k == KR - 1),
                            )
                    gsil = work.tile([128, C], BF16, tag="gsil",
                                     name=f"gsil_{i}")
                    nc.scalar.activation(
                        gsil[:], gps[:, :, :chunk],
                        mybir.ActivationFunctionType.Silu,
                    )
                    gsils[i] = gsil
                if i >= 1:
                    f = i - 1
                    ups = pmm.tile([128, nch, 512], F32, tag="mm",
                                   name=f"ups_{f}")
                    for k in range(KR):
                        for n in range(nch):
                            c0 = n * chunk
                            nc.tensor.matmul(
                                ups[:, n, :chunk], vu_f(f, k),
                                u1s[:, k, c0:c0 + chunk],
                                start=(k == 0), stop=(k == KR - 1),
                            )
                    nc.vector.tensor_mul(
                        afs[:, f, :], gsils[f][:], ups[:, :, :chunk]
                    )

            # --- phase B2: d1T [R, C] = Ud'.T @ aT, accumulated over all f
            # as a pure PE pass. m-major so the m=0 PSUM->SBUF copies run
            # under the m=1 matmul stream; only m=1's copies sit on the
            # B2->C boundary.
            for m in range(KR):
                d1p = pmm.tile([128, nch, 512], F32, tag="mm",
                               name=f"d1p_{m}")
                for n in range(nch):
                    c0 = n * chunk
                    for f in range(KF):
                        nc.tensor.matmul(
                            d1p[:, n, :chunk], udc_f(f, m),
                            afs[:, f, c0:c0 + chunk],
                            start=(f == 0), stop=(f == KF - 1),
                        )
                    # each column's copy overlaps the next chain's matmuls;
                    # the final (m1,n1) copy goes on the faster DVE path.
                    if n < nch - 1 or m < KR - 1:
                        nc.scalar.activation(
                            d1s[:, m, c0:c0 + chunk], d1p[:, n, :chunk],
                            mybir.ActivationFunctionType.Copy,
                        )
                    else:
                        nc.vector.tensor_copy(
                            d1s[:, m, c0:c0 + chunk], d1p[:, n, :chunk]
                        )

            # --- phase C: yT [H, C] = Vd.T @ d1T. PSUM->SBUF copies
            # alternate ACT/DVE; output DMAs alternate the two HWDGE rings
            # (SWDGE/gpsimd drains slowly at kernel end).
            out_engines = (nc.sync, nc.scalar)
            for m in range(MH):
                yps = pmm.tile([128, nch, 512], F32, tag="mm", name=f"yps_{m}")
                for n in range(nch):
                    for k in range(KR):
                        c0 = n * chunk
                        nc.tensor.matmul(
                            yps[:, n, :chunk],
                            vds[:, m * R + k * 128:m * R + (k + 1) * 128],
                            d1s[:, k, c0:c0 + chunk],
                            start=(k == 0), stop=(k == KR - 1),
                        )
                yts = work.tile([128, C], BF16, tag="yts", bufs=4,
                                name=f"yts_{m}")
                if m >= MH - 2:
                    # last piece: split copy + DMA across engines/rings to
                    # shorten the output tail after the final matmul.
                    for n in range(nch):
                        c0 = n * chunk
                        if n % 2 == 0:
                            nc.vector.tensor_copy(
                                yts[:, c0:c0 + chunk], yps[:, n, :chunk]
                            )
                        else:
                            nc.scalar.activation(
                                yts[:, c0:c0 + chunk], yps[:, n, :chunk],
                                mybir.ActivationFunctionType.Copy,
                            )
                        out_engines[n % 2].dma_start(
                            ytp[:, m * C + c0:m * C + c0 + chunk],
                            yts[:, c0:c0 + chunk],
                        )
                else:
                    if m % 2 == 0:
                        nc.vector.tensor_copy(yts[:], yps[:, :, :chunk])
                    else:
                        nc.scalar.activation(
                            yts[:], yps[:, :, :chunk],
                            mybir.ActivationFunctionType.Copy,
                        )
                    out_engines[m % 2].dma_start(
                        ytp[:, m * C:(m + 1) * C], yts[:]
                    )

    nc.finalize()
    return nc


def _pack_k(a, kt):
    """[kt*128, X] -> [128, kt, X] partition-tiled per k."""
    x = a.shape[1]
    return np.ascontiguousarray(a.reshape(kt, 128, x).transpose(1, 0, 2))


def _pack_fmajor(a, kt):
    """[kt*128, ft*128] -> [128, ft, kt*128]: f-major, k tiles adjacent."""
    ft = a.shape[1] // 128
    return np.ascontiguousarray(
        a.reshape(kt, 128, ft, 128).transpose(1, 2, 0, 3).reshape(128, ft, kt * 128)
    )


def kernel(hidden_states, gate_w, Ug, Cg, Vg, Uu, Cu, Vu, Ud, Cd, Vd):
    global LAST_RESULT
    hidden_states = np.asarray(hidden_states, dtype=np.float32)
    gate_w = np.asarray(gate_w, dtype=np.float32)
    b, s, h = hidden_states.shape
    x = hidden_states.reshape(-1, h)
    T = x.shape[0]

    # --- router (host; part of dispatch)
    logits = (x @ gate_w).astype(np.float64)
    lmax = logits.max(axis=-1, keepdims=True)
    p = np.exp(logits - lmax)
    p /= p.sum(axis=-1, keepdims=True)
    i1 = np.argmax(p, axis=-1)
    p1 = p[np.arange(T), i1]
    p_masked = p.copy()
    p_masked[np.arange(T), i1] = -np.inf
    i2 = np.argmax(p_masked, axis=-1)
    p2 = p[np.arange(T), i2]
    w1 = (p1 / (p1 + p2)).astype(np.float32)
    w2 = (p2 / (p1 + p2)).astype(np.float32)

    idx_e = []
    wgt_e = []
    for e in range(E):
        sel1 = np.nonzero(i1 == e)[0]
        sel2 = np.nonzero(i2 == e)[0]
        ids = np.concatenate([sel1, sel2])
        ws = np.concatenate([w1[sel1], w2[sel2]])
        idx_e.append(ids)
        wgt_e.append(ws)

    max_n = max(len(ids) for ids in idx_e)
    nch = max(1, -(-max_n // 512))
    chunk = -(-max_n // (nch * 4)) * 4
    C = nch * chunk

    key = (C, nch)
    if key not in _BUILD_CACHE:
        _BUILD_CACHE[key] = _build(C, nch)
    nc = _BUILD_CACHE[key]

    f32 = np.float32
    in_maps = []
    for e in range(E):
        ids = idx_e[e]
        xT = np.zeros((h, C), f32)
        xT[:, :len(ids)] = x[ids].T
        ugc = (Ug[e] @ Cg).astype(f32)
        uuc = (Uu[e] @ Cu).astype(f32)
        udc = (Ud[e] @ Cd).astype(f32)
        # abuf: per-k contiguous blocks [128, ugc_k | uuc_k | xt_k]
        abuf = np.ascontiguousarray(np.concatenate(
            [_pack_k(ugc, KH), _pack_k(uuc, KH), _pack_k(xT, KH)], axis=2
        ).transpose(1, 0, 2)).astype(BF16NP)  # [KH, 128, AB]
        # wbuf: per-f blocks [vg_f | vu_f | udc_f], paired per piece
        wflat = np.concatenate(
            [
                _pack_fmajor(np.asarray(Vg[e], f32), KR),
                _pack_fmajor(np.asarray(Vu[e], f32), KR),
                _pack_k(udc, KF),
            ],
            axis=2,
        ).transpose(1, 0, 2)  # [KF, 128, WB]
        wbuf = np.ascontiguousarray(
            wflat.reshape(KF // 2, 2, 128, wflat.shape[2])
            .transpose(0, 2, 1, 3)
            .reshape(KF // 2, 128, -1)
        ).astype(BF16NP)  # [NFP, 128, 2*WB]
        in_maps.append({
            "abuf": abuf,
            "wbuf": wbuf,
            "vdp": np.ascontiguousarray(
                _pack_fmajor(np.asarray(Vd[e], f32), KR).reshape(128, -1)
            ).astype(BF16NP),
        })

    res = run_bass_kernel_spmd(nc, in_maps, list(range(E)))
    LAST_RESULT = res

    out = np.zeros((T, h), f32)
    for e in range(E):
        ids = idx_e[e]
        ytp = np.asarray(res.results[e]["ytp"], dtype=f32)
        yT = ytp.reshape(128, MH, C).transpose(1, 0, 2).reshape(h, C)
        out[ids] += wgt_e[e][:, None] * yT[:, :len(ids)].T
    return out.reshape(b, s, h)


# revision 40
# speedup vs baseline: 1.0196x; 1.0196x over previous
"""Compressed MoE block on 8 Trainium2 NeuronCores.

Expert-parallel sharding: core e owns expert e. The router (tiny: T x H @
H x E) runs on host as part of dispatch; tokens are gathered per selected
expert (top-2), padded to a fixed capacity C, and each core runs the full
factored FFN chain for its expert in token-transposed layout:

    g1T = Ug'(e).T @ xT          (Ug' = Ug @ Cg folded on host)
    gT  = Vg(e).T  @ g1T
    u1T = Uu'(e).T @ xT
    uT  = Vu(e).T  @ u1T
    aT  = silu(gT) * uT
    d1T = Ud'(e).T @ aT          (Ud' = Ud @ Cd)
    yT  = Vd(e).T  @ d1T

Everything on device is bfloat16 (fp32 PSUM accumulate), which halves the
HBM input stream (~7 MB/core) and runs the PE at the full 1-cycle/column
rate. Phase B is split: B1 computes aT for all F tiles into SBUF with
gate/up PSUM tiles double-buffered across f (no PE stall on the ACT/DVE
consumers), then B2 accumulates d1T over all F tiles as a pure PE pass.
PSUM tiles span `nch` banks ([128, nch, 512] fp32) so the silu / mul /
PSUM->SBUF copies each cover all chunks in one wide instruction.
"""

import numpy as np
import ml_dtypes

import concourse.bacc as bacc
import concourse.mybir as mybir
import concourse.tile as tile
from concourse.bass_utils import run_bass_kernel_spmd

F32 = mybir.dt.float32
BF16 = mybir.dt.bfloat16
BF16NP = ml_dtypes.bfloat16

E = 8
KTOP = 2
H = 1024
FF = 2816
R = 256
KH = H // 128    # 8
KR = R // 128    # 2
KF = FF // 128   # 22
MH = H // 128    # 8

_BUILD_CACHE = {}
LAST_RESULT = None


def _build(C, nch):
    """Per-core bass program for capacity C split into nch chunks."""
    chunk = C // nch
    AB = 2 * R + C      # per-k block in abuf: [ugc_k | uuc_k | xt_k]
    WB = 3 * R          # per-f block in wbuf: [vg_f | vu_f | udc_f]
    NFP = (KF + 1) // 2  # f-pair DMA pieces
    nc = bacc.Bacc()

    abuf = nc.declare_dram_parameter("abuf", [KH, 128, AB], BF16, isOutput=False)
    wbuf = nc.declare_dram_parameter("wbuf", [NFP, 128, 2 * WB], BF16, isOutput=False)
    vdp = nc.declare_dram_parameter("vdp", [128, MH * R], BF16, isOutput=False)
    ytp = nc.declare_dram_parameter("ytp", [128, MH * C], BF16, isOutput=True)

    with tile.TileContext(nc) as tc:
        with (
            tc.tile_pool(name="wsb", bufs=1) as wsb,
            tc.tile_pool(name="work", bufs=3) as work,
            tc.tile_pool(name="pmm", bufs=8 // nch, space="PSUM") as pmm,
        ):
            ab = wsb.tile([128, KH, AB], BF16, tag="ab")
            wb = wsb.tile([128, KF, WB], BF16, tag="wb")
            vds = wsb.tile([128, MH * R], BF16, tag="vds")
            g1s = wsb.tile([128, KR, C], BF16, tag="g1s")
            u1s = wsb.tile([128, KR, C], BF16, tag="u1s")
            d1s = wsb.tile([128, KR, C], BF16, tag="d1s")
            afs = wsb.tile([128, KF, C], BF16, tag="afs")
            warm0 = wsb.tile([128, 256], F32, tag="warm0")
            warm = wsb.tile([128, 256], BF16, tag="warm")
            wsil = wsb.tile([128, 16], BF16, tag="wsil")

            def ugc_k(k, m):
                o = m * 128
                return ab[:, k, o:o + 128]

            def uuc_k(k, m):
                o = R + m * 128
                return ab[:, k, o:o + 128]

            def xt_k(k, c0):
                o = 2 * R + c0
                return ab[:, k, o:o + chunk]

            def vg_f(f, k):
                return wb[:, f, k * 128:(k + 1) * 128]

            def vu_f(f, k):
                return wb[:, f, R + k * 128:R + (k + 1) * 128]

            def udc_f(f, m):
                return wb[:, f, 2 * R + m * 128:2 * R + (m + 1) * 128]

            # --- PE warm-up: spin the PE from t~0 until the first abuf block
            # lands so the DVFS ramp completes during the DMA latency, and
            # preload the Silu table on ACT.
            nc.vector.memset(warm0[:], 0.0)
            nc.vector.tensor_copy(warm[:], warm0[:])
            nc.scalar.activation(
                wsil[:], warm[:, :16], mybir.ActivationFunctionType.Silu
            )
            wps = pmm.tile([128, nch, 512], F32, tag="mm", name="wps")
            NWARM = 8
            for i in range(NWARM):
                nc.tensor.matmul(
                    wps[:, 0, :16], warm[:, :128], warm[:, :16],
                    start=(i == 0), stop=(i == NWARM - 1),
                )

            # --- input DMAs. abuf alternates between the SP and ACT HWDGE
            # rings; wbuf + vdp follow on the SP ring (the ACT queue stays
            # free for silu work).
            for k in range(KH):
                eng = nc.sync if k % 2 == 0 else nc.scalar
                eng.dma_start(ab[:, k, :], abuf[k])
            for i in range(NFP):
                nc.sync.dma_start(wb[:, 2 * i:2 * i + 2, :], wbuf[i])
            nc.sync.dma_start(vds[:], vdp[:])

            # --- phase A: g1T/u1T [R, C] = Ug'/Uu'.T @ xT. k-outer with all
            # (tensor, m, chunk) accumulators concurrent, pacing the k-block
            # DMA stream; PSUM->SBUF copies at the end split across ACT/DVE.
            psA = [
                pmm.tile([128, nch, 512], F32, tag="mm", name=f"psA_{t}_{m}")
                for t in range(2) for m in range(KR)
            ]
            for k in range(KH):
                for t, wfun in enumerate((ugc_k, uuc_k)):
                    for m in range(KR):
                        for n in range(nch):
                            nc.tensor.matmul(
                                psA[t * KR + m][:, n, :chunk],
                                wfun(k, m),
                                xt_k(k, n * chunk),
                                start=(k == 0), stop=(k == KH - 1),
                            )
            for m in range(KR):
                nc.scalar.activation(
                    g1s[:, m, :], psA[m][:, :, :chunk],
                    mybir.ActivationFunctionType.Copy,
                )
            for m in range(KR):
                nc.vector.tensor_copy(u1s[:, m, :], psA[KR + m][:, :, :chunk])

            # --- phase B1: aT[f] = silu(Vg_f.T @ g1T) * (Vu_f.T @ u1T) for
            # all f, staged to SBUF. Software-pipelined with a one-f
            # stagger: each step runs gate matmuls for f=i and up matmuls
            # for f=i-1, so silu(i) (ACT) and mul(i-1) (DVE) overlap PE work
            # and PSUM banks recycle without stalling the PE.
            gsils = [None] * KF
            for i in range(KF + 1):
                if i < KF:
                    gps = pmm.tile([128, nch, 512], F32, tag="mm",
                                   name=f"gps_{i}")
                    for k in range(KR):
                        for n in range(nch):
                            c0 = n * chunk
                            nc.tensor.matmul(
                                gps[:, n, :chunk], vg_f(i, k),
                                g1s[:, k, c0:c0 + chunk],
                                start=(k == 0), stop=(k == KR - 1),
                            )
                    gsil = work.tile([128, C], BF16, tag="gsil",
                                     name=f"gsil_{i}")
                    nc.scalar.activation(
                        gsil[:], gps[:, :, :chunk],
                        mybir.ActivationFunctionType.Silu,
                    )
                    gsils[i] = gsil
                if i >= 1:
                    f = i - 1
                    ups = pmm.tile([128, nch, 512], F32, tag="mm",
                                   name=f"ups_{f}")
                    for k in range(KR):
                        for n in range(nch):
                            c0 = n * chunk
                            nc.tensor.matmul(
                                ups[:, n, :chunk], vu_f(f, k),
                                u1s[:, k, c0:c0 + chunk],
                                start=(k == 0), stop=(k == KR - 1),
                            )
                    nc.vector.tensor_mul(
                        afs[:, f, :], gsils[f][:], ups[:, :, :chunk]
                    )

            # --- phase B2: d1T [R, C] = Ud'.T @ aT, accumulated over all f
            # as a pure PE pass. m-major so the m=0 PSUM->SBUF copies run
            # under the m=1 matmul stream; only m=1's copies sit on the
            # B2->C boundary.
            for m in range(KR):
                d1p = pmm.tile([128, nch, 512], F32, tag="mm",
                               name=f"d1p_{m}")
                for f in range(KF):
                    for n in range(nch):
                        c0 = n * chunk
                        nc.tensor.matmul(
                            d1p[:, n, :chunk], udc_f(f, m),
                            afs[:, f, c0:c0 + chunk],
                            start=(f == 0), stop=(f == KF - 1),
                        )
                for n in range(nch):
                    c0 = n * chunk
                    # m=1's n=0 column is the first one phase C waits on:
                    # put it on the faster DVE path.
                    if (n + m) % 2 == 0:
                        nc.scalar.activation(
                            d1s[:, m, c0:c0 + chunk], d1p[:, n, :chunk],
                            mybir.ActivationFunctionType.Copy,
                        )
                    else:
                        nc.vector.tensor_copy(
                            d1s[:, m, c0:c0 + chunk], d1p[:, n, :chunk]
                        )

            # --- phase C: yT [H, C] = Vd.T @ d1T. PSUM->SBUF copies
            # alternate ACT/DVE; output DMAs alternate the two HWDGE rings
            # (SWDGE/gpsimd drains slowly at kernel end).
            out_engines = (nc.sync, nc.scalar)
            for m in range(MH):
                yps = pmm.tile([128, nch, 512], F32, tag="mm", name=f"yps_{m}")
                # k-outer: with B2 m-major, all k=0 operands (d1 m-tile 0)
                # are ready early; defer the k=1 matmuls behind them.
                for k in range(KR):
                    for n in range(nch):
                        c0 = n * chunk
                        nc.tensor.matmul(
                            yps[:, n, :chunk],
                            vds[:, m * R + k * 128:m * R + (k + 1) * 128],
                            d1s[:, k, c0:c0 + chunk],
                            start=(k == 0), stop=(k == KR - 1),
                        )
                yts = work.tile([128, C], BF16, tag="yts", bufs=4,
                                name=f"yts_{m}")
                if m >= MH - 2:
                    # last piece: split copy + DMA across engines/rings to
                    # shorten the output tail after the final matmul.
                    for n in range(nch):
                        c0 = n * chunk
                        if n % 2 == 0:
                            nc.vector.tensor_copy(
                                yts[:, c0:c0 + chunk], yps[:, n, :chunk]
                            )
                        else:
                            nc.scalar.activation(
                                yts[:, c0:c0 + chunk], yps[:, n, :chunk],
                                mybir.ActivationFunctionType.Copy,
                            )
                        out_engines[n % 2].dma_start(
                            ytp[:, m * C + c0:m * C + c0 + chunk],
                            yts[:, c0:c0 + chunk],
                        )
                else:
                    if m % 2 == 0:
                        nc.vector.tensor_copy(yts[:], yps[:, :, :chunk])
                    else:
                        nc.scalar.activation(
                            yts[:], yps[:, :, :chunk],
                            mybir.ActivationFunctionType.Copy,
                        )
                    out_engines[m % 2].dma_start(
                        ytp[:, m * C:(m + 1) * C], yts[:]
                    )

    nc.finalize()
    return nc


def _pack_k(a, kt):
    """[kt*128, X] -> [128, kt, X] partition-tiled per k."""
    x = a.shape[1]
    return np.ascontiguousarray(a.reshape(kt, 128, x).transpose(1, 0, 2))


def _pack_fmajor(a, kt):
    """[kt*128, ft*128] -> [128, ft, kt*128]: f-major, k tiles adjacent."""
    ft = a.shape[1] // 128
    return np.ascontiguousarray(
        a.reshape(kt, 128, ft, 128).transpose(1, 2, 0, 3).reshape(128, ft, kt * 128)
    )


def kernel(hidden_states, gate_w, Ug, Cg, Vg, Uu, Cu, Vu, Ud, Cd, Vd):
    global LAST_RESULT
    hidden_states = np.asarray(hidden_states, dtype=np.float32)
    gate_w = np.asarray(gate_w, dtype=np.float32)
    b, s, h = hidden_states.shape
    x = hidden_states.reshape(-1, h)
    T = x.shape[0]

    # --- router (host; part of dispatch)
    logits = (x @ gate_w).astype(np.float64)
    lmax = logits.max(axis=-1, keepdims=True)
    p = np.exp(logits - lmax)
    p /= p.sum(axis=-1, keepdims=True)
    i1 = np.argmax(p, axis=-1)
    p1 = p[np.arange(T), i1]
    p_masked = p.copy()
    p_masked[np.arange(T), i1] = -np.inf
    i2 = np.argmax(p_masked, axis=-1)
    p2 = p[np.arange(T), i2]
    w1 = (p1 / (p1 + p2)).astype(np.float32)
    w2 = (p2 / (p1 + p2)).astype(np.float32)

    idx_e = []
    wgt_e = []
    for e in range(E):
        sel1 = np.nonzero(i1 == e)[0]
        sel2 = np.nonzero(i2 == e)[0]
        ids = np.concatenate([sel1, sel2])
        ws = np.concatenate([w1[sel1], w2[sel2]])
        idx_e.append(ids)
        wgt_e.append(ws)

    max_n = max(len(ids) for ids in idx_e)
    nch = max(1, -(-max_n // 512))
    chunk = -(-max_n // (nch * 4)) * 4
    C = nch * chunk

    key = (C, nch)
    if key not in _BUILD_CACHE:
        _BUILD_CACHE[key] = _build(C, nch)
    nc = _BUILD_CACHE[key]

    f32 = np.float32
    in_maps = []
    for e in range(E):
        ids = idx_e[e]
        xT = np.zeros((h, C), f32)
        xT[:, :len(ids)] = x[ids].T
        ugc = (Ug[e] @ Cg).astype(f32)
        uuc = (Uu[e] @ Cu).astype(f32)
        udc = (Ud[e] @ Cd).astype(f32)
        # abuf: per-k contiguous blocks [128, ugc_k | uuc_k | xt_k]
        abuf = np.ascontiguousarray(np.concatenate(
            [_pack_k(ugc, KH), _pack_k(uuc, KH), _pack_k(xT, KH)], axis=2
        ).transpose(1, 0, 2)).astype(BF16NP)  # [KH, 128, AB]
        # wbuf: per-f blocks [vg_f | vu_f | udc_f], paired per piece
        wflat = np.concatenate(
            [
                _pack_fmajor(np.asarray(Vg[e], f32), KR),
                _pack_fmajor(np.asarray(Vu[e], f32), KR),
                _pack_k(udc, KF),
            ],
            axis=2,
        ).transpose(1, 0, 2)  # [KF, 128, WB]
        wbuf = np.ascontiguousarray(
            wflat.reshape(KF // 2, 2, 128, wflat.shape[2])
            .transpose(0, 2, 1, 3)
            .reshape(KF // 2, 128, -1)
        ).astype(BF16NP)  # [NFP, 128, 2*WB]
        in_maps.append({
            "abuf": abuf,
            "wbuf": wbuf,
            "vdp": np.ascontiguousarray(
                _pack_fmajor(np.asarray(Vd[e], f32), KR).reshape(128, -1)
            ).astype(BF16NP),
        })

    res = run_bass_kernel_spmd(nc, in_maps, list(range(E)))
    LAST_RESULT = res

    out = np.zeros((T, h), f32)
    for e in range(E):
        ids = idx_e[e]
        ytp = np.asarray(res.results[e]["ytp"], dtype=f32)
        yT = ytp.reshape(128, MH, C).transpose(1, 0, 2).reshape(h, C)
        out[ids] += wgt_e[e][:, None] * yT[:, :len(ids)].T
    return out.reshape(b, s, h)
